# revision 34
# baseline (speedup 1.0000x reference)
"""CoAttention Trainium2 kernel.

Problem: B=16, PLEN=1024, QLEN=256, D=256 fp32.
  score[b,p,q] = passage.w_p + question.w_q + (passage*w_pq).question + b
  masked-softmax both ways, three attention matmuls.

Data-parallel over batch across 8 NeuronCores (2 batches/core), everything
per-batch local.

Math (per batch), with E0 = exp(S0), S0[p,q] = (P w_pq)·Q:
  g[q]     = exp(sq[q] + b - 1e7*qm[q])
  ET'[q,p] = E0[p,q]^T * g[q]        (PE transpose of E0, g rides the drain)
  h[p]     = exp(sp[p]) * (1-pm[p])
  p2q[p,:] = (ET'^T @ [Q|1]) * kp/dp   (dp from the ones column)
  q2p[q,:] = (E0 @ [P*h|h]) / dq       (dq from the h column)
  coatt    = (ET'^T @ q2p) * kp/dp
Row-constant softmax factors cancel; masks enter as exact zeros in g/h.

Engine plan: all matmul operands are bf16 (1 cyc/row); E0 is written by
ACT exp directly as bf16; ET' comes from PE transposes of E0 instead of a
second score matmul + exp. S0 tiles pair two p-tiles per PSUM bank so
each exp is 512 wide. Outputs are stored to HBM as bf16 (halves output
DMA) and widened to fp32 on the host. PSUM->SBUF drains are split
ACT/DVE; bf16 conversions and small SBUF ops go to Pool; inputs stream on
the SP DMA queue, outputs on SP (p2q) and Pool (coatt), one tile per DMA.

The container's walrus accepts only ONE sync-wait per non-matmul
instruction (and none on matmuls); a BIR post-pass splits waits into
single-wait EventSemaphore carriers. All matmul moving dims are even.
"""

import numpy as np
import orjson

import concourse.bass as bass
import concourse.mybir as mybir
import concourse.tile as tile
from concourse.bass_utils import run_bass_kernel_spmd
from concourse.masks import make_identity

F32 = mybir.dt.float32
F32R = mybir.dt.float32r
BF16 = mybir.dt.bfloat16
I32 = mybir.dt.int32
AF = mybir.ActivationFunctionType
ALU = mybir.AluOpType

N_CORES = 8
B, PLEN, QLEN, D = 16, 1024, 256, 256
NB = B // N_CORES  # batches per core
PT_T = PLEN // 128  # 8 p-tiles
QT_T = QLEN // 128  # 2 q-tiles
DT_T = D // 128  # 2 d-tiles
MASK = -10000000.0

# ---------------------------------------------------------------------------
# walrus single-wait workaround


def _split_waits_in_bir(bir: dict) -> None:
    for f in bir.get("functions", []):
        for blk in f.get("blocks", []):
            out = []
            for i in blk.get("instructions", []):
                si = i.get("sync_info")
                ow = (si or {}).get("on_wait") or []
                limit = 0 if i.get("opcode") == "Matmult" else 1
                if len(ow) > limit:
                    for k, w in enumerate(ow[limit:]):
                        out.append(
                            {
                                "debug": i.get("debug"),
                                "engine": i["engine"],
                                "ins": [],
                                "outs": [],
                                "name": f"{i['name']}__w{k}",
                                "opcode": "EventSemaphore",
                                "sync_info": {"on_update": [], "on_wait": [w]},
                            }
                        )
                    si["on_wait"] = ow[:limit]
                out.append(i)
            blk["instructions"] = out


_patched = False


def _install_bir_wait_split():
    global _patched
    if _patched:
        return
    _patched = True
    import concourse.bass2jax as b2j
    import concourse.bass_utils as bu

    orig = bu.compile_bir_kernel

    def patched(bir_json, tmpdir, neff_name="file.neff"):
        bir = orjson.loads(bir_json)
        _split_waits_in_bir(bir)
        return orig(orjson.dumps(bir), tmpdir, neff_name)

    bu.compile_bir_kernel = patched
    b2j.compile_bir_kernel = patched


# ---------------------------------------------------------------------------


def build_nc(bufs_cfg=None) -> bass.Bass:
    cfg = {"tp": 2, "s0": 2, "at": 3, "mi": 1, "big": 2, "small": 2}
    if bufs_cfg:
        cfg.update(bufs_cfg)
    nc = bass.Bass()
    passage = nc.declare_dram_parameter("passage", [NB, PLEN, D], F32, isOutput=False)
    question = nc.declare_dram_parameter("question", [NB, QLEN, D], F32, isOutput=False)
    pmask = nc.declare_dram_parameter("passage_mask", [NB, PLEN], I32, isOutput=False)
    qmask = nc.declare_dram_parameter("question_mask", [NB, QLEN], I32, isOutput=False)
    w_all = nc.declare_dram_parameter("W", [3 * D], F32, isOutput=False)
    b_in = nc.declare_dram_parameter("b", [1], F32, isOutput=False)
    out_p2q = nc.declare_dram_parameter("p2q", [NB, PLEN, D], BF16, isOutput=True)
    out_co = nc.declare_dram_parameter("coatt", [NB, PLEN, D], BF16, isOutput=True)

    with tile.TileContext(nc) as tc:
        with (
            tc.tile_pool(name="const", bufs=1) as const_pool,
            tc.tile_pool(name="big", bufs=cfg["big"]) as big,
            tc.tile_pool(name="small", bufs=cfg["small"]) as small,
            tc.tile_pool(name="tp_ps", bufs=cfg["tp"], space="PSUM") as tp_ps,
            tc.tile_pool(name="s0_ps", bufs=cfg["s0"], space="PSUM") as s0_ps,
            tc.tile_pool(name="at_ps", bufs=cfg["at"], space="PSUM") as at_ps,
            tc.tile_pool(name="mi_ps", bufs=cfg["mi"], space="PSUM") as mi_ps,
        ):
            # ---- ACT exp table warm-up (off the critical path) ----------
            warm_in = const_pool.tile([128, 2], F32, name="warm_in")
            nc.gpsimd.memset(warm_in[:], 0.0)
            warm_out = const_pool.tile([128, 2], F32, name="warm_out")
            nc.scalar.activation(warm_out[:], warm_in[:], AF.Exp)

            ident = const_pool.tile([128, 128], F32, name="ident")
            make_identity(nc, ident[:])
            ident_b_t = const_pool.tile([128, 128], BF16, name="ident_b_t")
            nc.gpsimd.tensor_copy(ident_b_t[:], ident[:])
            ident_b = ident_b_t[:]

            onesf = const_pool.tile([128, 2], F32, name="onesf")
            nc.gpsimd.memset(onesf[:], 1.0)
            ones_b = const_pool.tile([128, 2], BF16, name="ones_b")
            nc.vector.tensor_copy(ones_b[:], onesf[:])

            # weight columns: [d_in_tile, k]  cols: wp0 wp1 wq0 wq1 wpq0 wpq1
            # (w6 rides the SP queue FIRST: it gates qwt and the first S0)
            w6 = const_pool.tile([128, 6], F32, name="w6")
            nc.sync.dma_start(w6[:], w_all[:].rearrange("(k d) -> d k", d=128))
            w_pq = w6[:, 2 * DT_T : 3 * DT_T]
            # duplicated 2-wide w_p columns for the tiny sp matmuls
            w_p_r = const_pool.tile([128, DT_T, 2], BF16, name="w_p_r")
            for j in range(DT_T):
                for k in range(2):
                    nc.vector.tensor_copy(w_p_r[:, j, k : k + 1], w6[:, j : j + 1])
            # w_q broadcast across partitions, for the sq row-reduce
            w_q_bc = const_pool.tile([128, D], F32, name="w_q_bc")
            pm_all = const_pool.tile([128, NB, PT_T], I32, name="pm_all")
            qm_all = const_pool.tile([128, NB, QT_T], I32, name="qm_all")
            b_sb = const_pool.tile([128, 1], F32, name="b_sb")

            def emit_const_loads():
                # SP queue, after the input DMAs: masks/bias are needed
                # later than Q/P, and SP FIFO order is issue order.
                nc.sync.dma_start(
                    w_q_bc[:], w_all[D : 2 * D].partition_broadcast(128)
                )
                nc.sync.dma_start(
                    pm_all[:], pmask[:].rearrange("n (t p) -> p n t", p=128)
                )
                nc.sync.dma_start(
                    qm_all[:], qmask[:].rearrange("n (t q) -> q n t", q=128)
                )
                nc.sync.dma_start(b_sb[:], b_in[0:1].partition_broadcast(128))

            def emit_batch(bi):
                p2q_dst = out_p2q[bi].rearrange("(t p) d -> p t d", p=128)
                co_dst = out_co[bi].rearrange("(t p) d -> p t d", p=128)

                # ---- phase: input DMAs (SP queue) -----------------------
                q_sb = small.tile([128, QT_T, D], F32, name="q_sb", tag="q_sb")
                q_src = question[bi].rearrange("(t q) d -> q t d", q=128)
                p_sb = big.tile([128, PT_T, D], F32, name="p_sb", tag="p_sb")
                p_src = passage[bi].rearrange("(t p) d -> p t d", p=128)
                nc.sync.dma_start(q_sb[:], q_src[:])
                nc.sync.dma_start(p_sb[:, 0:4, :], p_src[:, 0:4, :])
                yield "dma_a"
                nc.sync.dma_start(p_sb[:, 4:8, :], p_src[:, 4:8, :])
                yield "dma_b"

                # ---- phase: head (masks, Q transposes, sq, g, qb) -------
                pm_f = small.tile([128, PT_T], F32, name="pm_f", tag="pm_f")
                nc.vector.tensor_copy(pm_f[:], pm_all[:, bi])
                kp = small.tile([128, PT_T], F32, name="kp", tag="kp")
                nc.vector.tensor_scalar(kp[:], pm_f[:], -1.0, 1.0, ALU.mult, ALU.add)
                qm_f = small.tile([128, QT_T], F32, name="qm_f", tag="qm_f")
                nc.vector.tensor_copy(qm_f[:], qm_all[:, bi])
                # qmb = qm*MASK + b
                qmb = small.tile([128, QT_T], F32, name="qmb", tag="qmb")
                nc.vector.tensor_scalar(
                    qmb[:], qm_f[:], MASK, b_sb[:, 0:1], ALU.mult, ALU.add
                )

                # qb = [Q*g | g | g] in bf16, [q_part, tq, 258] (Pool builds;
                # folding g here makes ET' a plain transpose of E0)
                qb = small.tile([128, QT_T, QLEN + 2], BF16, name="qb", tag="qb")
                # plain bf16 Q for the score transposes
                q_bp = small.tile([128, QT_T, QLEN], BF16, name="q_bp", tag="q_bp")
                for t4 in range(QT_T):
                    nc.gpsimd.tensor_copy(q_bp[:, t4, :], q_sb[:, t4, :])

                # Q transposes (bf16): one bank holds both j d-tiles
                qwt = small.tile([128, DT_T, QLEN], BF16, name="qwt", tag="qwt")
                tq = tp_ps.tile([128, 512], BF16, name="tq", tag="tp")
                for j in range(DT_T):
                    for t4 in range(QT_T):
                        nc.tensor.transpose(
                            tq[:, j * 256 + t4 * 128 : j * 256 + (t4 + 1) * 128],
                            q_bp[:, t4, j * 128 : (j + 1) * 128],
                            ident_b,
                        )
                for j in range(DT_T):
                    # QwT = QT * w_pq (per-partition d scale), bf16
                    nc.vector.tensor_scalar_mul(
                        qwt[:, j, :], tq[:, j * 256 : (j + 1) * 256],
                        w_pq[:, j : j + 1],
                    )

                # sq by row-reduce: sq[q] = sum_d Q[q,d] * w_q[d]
                junk = small.tile([128, D], BF16, name="junk", tag="junk")
                sq_sb = small.tile([128, QT_T], F32, name="sq_sb", tag="sq_sb")
                for t4 in range(QT_T):
                    nc.vector.tensor_tensor_reduce(
                        junk[:],
                        q_sb[:, t4, :],
                        w_q_bc[:],
                        1.0,
                        0.0,
                        ALU.mult,
                        ALU.add,
                        sq_sb[:, t4 : t4 + 1],
                    )
                # g = exp(sq + qm*MASK + b) per q-partition, [128, QT_T]
                gb = small.tile([128, QT_T], F32, name="gb", tag="gb")
                nc.vector.tensor_add(gb[:], sq_sb[:], qmb[:])
                g = small.tile([128, QT_T], F32, name="g", tag="g")
                nc.scalar.activation(g[:], gb[:], AF.Exp)
                for t4 in range(QT_T):
                    nc.gpsimd.tensor_scalar_mul(
                        qb[:, t4, 0:QLEN], q_sb[:, t4, :], g[:, t4 : t4 + 1]
                    )
                    nc.gpsimd.tensor_copy(
                        qb[:, t4, QLEN : QLEN + 1], g[:, t4 : t4 + 1]
                    )
                    nc.gpsimd.tensor_copy(
                        qb[:, t4, QLEN + 1 : QLEN + 2], g[:, t4 : t4 + 1]
                    )

                # p_b: bf16 copy of P for transposes (Pool)
                p_b = big.tile([128, PT_T, D], BF16, name="p_b", tag="p_b")
                for t in range(PT_T):
                    nc.gpsimd.tensor_copy(p_b[:, t, :], p_sb[:, t, :])
                yield "head"

                # ---- per-batch big tiles --------------------------------
                pt_b = big.tile([128, DT_T, PLEN], BF16, name="pt_b", tag="pt_b")
                # E0 pairs: [p_part, pair, 512] bf16 (tile 2u in cols 0:256)
                e_sb = big.tile([128, PT_T // 2, 512], BF16, name="e_sb", tag="e_sb")
                et_sb = big.tile([128, QT_T, PLEN], BF16, name="et_sb", tag="et_sb")
                ph = big.tile([128, PT_T, D + 2], BF16, name="ph", tag="ph")
                q2p = small.tile([128, QT_T, D], BF16, name="q2p", tag="q2p")
                p2q_sb = big.tile([128, PT_T, D], BF16, name="p2q_sb", tag="p2q_sb")
                co_sb = big.tile([128, PT_T, D], BF16, name="co_sb", tag="co_sb")
                rp = small.tile([128, PT_T], F32, name="rp", tag="rp")
                # misc PSUM bank: sp only (released at the h exp)
                mi = mi_ps.tile([128, 2 * PT_T], F32, name="mi", tag="mi")
                sp = mi[:]

                def e_chunk(t, tq_i):
                    # E0 column chunk [p_part, 128] for (p-tile t, q-tile tq_i)
                    c = (t % 2) * 256 + tq_i * 128
                    return e_sb[:, t // 2, c : c + 128]

                def emit_ap(t):
                    # p2q attention for tile t (K = q over 2 q-tiles)
                    ap_ = at_ps.tile([128, QLEN + 2], F32, name="ap_", tag="at")
                    for tq_i in range(QT_T):
                        nc.tensor.matmul(
                            ap_[:],
                            et_sb[:, tq_i, t * 128 : (t + 1) * 128],
                            qb[:, tq_i, :],
                            start=(tq_i == 0),
                            stop=(tq_i == QT_T - 1),
                        )
                    v2 = small.tile([128, 1], F32, name="v2", tag="v2")
                    nc.vector.reciprocal(v2[:], ap_[:, QLEN : QLEN + 1])
                    nc.vector.tensor_mul(rp[:, t : t + 1], v2[:], kp[:, t : t + 1])
                    if t % 2 == 0:
                        nc.scalar.activation(
                            p2q_sb[:, t, :], ap_[:, 0:QLEN], AF.Copy,
                            scale=rp[:, t : t + 1],
                        )
                    else:
                        nc.vector.tensor_scalar_mul(
                            p2q_sb[:, t, :], ap_[:, 0:QLEN], rp[:, t : t + 1]
                        )
                    if t == 3:
                        nc.sync.dma_start(p2q_dst[:, 0:4, :], p2q_sb[:, 0:4, :])
                    elif t == 5:
                        nc.sync.dma_start(p2q_dst[:, 4:6, :], p2q_sb[:, 4:6, :])
                    elif t == 7:
                        nc.sync.dma_start(p2q_dst[:, 6:8, :], p2q_sb[:, 6:8, :])

                def emit_co_pair(t0):
                    cob = at_ps.tile([128, 512], F32, name="cob", tag="at")
                    for half in range(2):
                        t = t0 + half
                        for tq_i in range(QT_T):
                            nc.tensor.matmul(
                                cob[:, half * 256 : (half + 1) * 256],
                                et_sb[:, tq_i, t * 128 : (t + 1) * 128],
                                q2p[:, tq_i, :],
                                start=(tq_i == 0),
                                stop=(tq_i == QT_T - 1),
                            )
                    for half in range(2):
                        t = t0 + half
                        if half == 1:
                            nc.scalar.activation(
                                co_sb[:, t, :],
                                cob[:, half * 256 : (half + 1) * 256],
                                AF.Copy,
                                scale=rp[:, t : t + 1],
                            )
                        else:
                            nc.vector.tensor_scalar_mul(
                                co_sb[:, t, :],
                                cob[:, half * 256 : (half + 1) * 256],
                                rp[:, t : t + 1],
                            )
                    if t0 == 2:
                        nc.sync.dma_start(co_dst[:, 0:4, :], co_sb[:, 0:4, :])
                    elif t0 == 4:
                        nc.sync.dma_start(co_dst[:, 4:6, :], co_sb[:, 4:6, :])
                    elif t0 == 6:
                        nc.sync.dma_start(co_dst[:, 6:8, :], co_sb[:, 6:8, :])

                def emit_grp_scores(grp):
                    t_lo = grp * 4
                    # PT transposes for this half (per-j bank of 4 tiles)
                    for j in range(DT_T):
                        tp = tp_ps.tile([128, 512], BF16, name="tp", tag="tp")
                        for t4 in range(4):
                            t = t_lo + t4
                            nc.tensor.transpose(
                                tp[:, t4 * 128 : (t4 + 1) * 128],
                                p_b[:, t, j * 128 : (j + 1) * 128],
                                ident_b,
                            )
                        if j == 0:
                            nc.vector.tensor_copy(
                                pt_b[:, j, grp * 512 : (grp + 1) * 512], tp[:]
                            )
                        else:
                            nc.scalar.copy(
                                pt_b[:, j, grp * 512 : (grp + 1) * 512], tp[:]
                            )
                    # S0 pairs -> paired exp -> bf16 E0; sp rides along
                    for pair in range(2):
                        t0 = t_lo + 2 * pair
                        s0b = s0_ps.tile([128, 512], F32, name="s0b", tag="s0")
                        for half in range(2):
                            t = t0 + half
                            for j in range(DT_T):
                                nc.tensor.matmul(
                                    s0b[:, half * 256 : (half + 1) * 256],
                                    pt_b[:, j, t * 128 : (t + 1) * 128],
                                    qwt[:, j, :],
                                    start=(j == 0),
                                    stop=(j == DT_T - 1),
                                )
                            for j in range(DT_T):
                                nc.tensor.matmul(
                                    sp[:, 2 * t : 2 * t + 2],
                                    pt_b[:, j, t * 128 : (t + 1) * 128],
                                    w_p_r[:, j, :],
                                    start=(j == 0),
                                    stop=(j == DT_T - 1),
                                )
                        nc.scalar.activation(e_sb[:, t0 // 2, :], s0b[:], AF.Exp)
                    # ET' = E0^T * g via PE transposes of E0 chunks
                    # (both q-tiles in one 2KB bank from the s0 pool)
                    etp = s0_ps.tile([128, 1024], BF16, name="etp", tag="s0")
                    for tq_i in range(QT_T):
                        for t4 in range(4):
                            t = t_lo + t4
                            nc.tensor.transpose(
                                etp[:, tq_i * 512 + t4 * 128 :
                                     tq_i * 512 + (t4 + 1) * 128],
                                e_chunk(t, tq_i),
                                ident_b,
                            )
                    for tq_i in range(QT_T):
                        if tq_i == 0:
                            nc.vector.tensor_copy(
                                et_sb[:, tq_i, grp * 512 : (grp + 1) * 512],
                                etp[:, 0:512],
                            )
                        else:
                            nc.scalar.copy(
                                et_sb[:, tq_i, grp * 512 : (grp + 1) * 512],
                                etp[:, 512:1024],
                            )

                # ---- phase g0s/g0a: first half scores + its p2q ---------
                emit_grp_scores(0)
                yield "g0s"
                for t in range(0, 4):
                    emit_ap(t)
                yield "g0a"

                # ---- phases g1s/hph: second half scores + h/ph ----------
                emit_grp_scores(1)
                yield "g1s"
                # h = exp(sp) * kp ; ph = [P*h | h | h] bf16
                h_raw = small.tile([128, 2 * PT_T], F32, name="h_raw", tag="h_raw")
                nc.scalar.activation(h_raw[:], sp[:], AF.Exp)
                hk = small.tile([128, 2 * PT_T], F32, name="hk", tag="hk")
                for t in range(PT_T):
                    nc.gpsimd.tensor_mul(
                        hk[:, 2 * t : 2 * t + 1],
                        h_raw[:, 2 * t : 2 * t + 1],
                        kp[:, t : t + 1],
                    )
                for t in range(PT_T):
                    nc.gpsimd.tensor_scalar_mul(
                        ph[:, t, 0:D], p_sb[:, t, :], hk[:, 2 * t : 2 * t + 1]
                    )
                    nc.gpsimd.tensor_copy(ph[:, t, D : D + 1], hk[:, 2 * t : 2 * t + 1])
                    nc.gpsimd.tensor_copy(
                        ph[:, t, D + 1 : D + 2], hk[:, 2 * t : 2 * t + 1]
                    )
                yield "hph"

                # ---- phase aq: q2p attention ----------------------------
                for tq_i in range(QT_T):
                    aq = at_ps.tile([128, D + 2], F32, name="aq", tag="at")
                    for t in range(PT_T):
                        nc.tensor.matmul(
                            aq[:],
                            e_chunk(t, tq_i),
                            ph[:, t, :],
                            start=(t == 0),
                            stop=(t == PT_T - 1),
                        )
                    u2 = small.tile([128, 1], F32, name="u2", tag="u2")
                    nc.vector.reciprocal(u2[:], aq[:, D : D + 1])
                    # co consumes ET (no g), so q2p carries the g factor
                    gu = small.tile([128, 1], F32, name="gu", tag="gu")
                    nc.vector.tensor_mul(gu[:], u2[:], g[:, tq_i : tq_i + 1])
                    if tq_i == 0:
                        nc.vector.tensor_scalar_mul(q2p[:, tq_i, :], aq[:, 0:D], gu[:])
                    else:
                        nc.scalar.activation(
                            q2p[:, tq_i, :], aq[:, 0:D], AF.Copy, scale=gu[:]
                        )
                yield "aq"

                # ---- tail: remaining p2q + coattention ------------------
                emit_ap(4)
                emit_ap(5)
                emit_co_pair(0)
                yield "apco1"
                emit_ap(6)
                emit_ap(7)
                emit_co_pair(2)
                yield "apco2"
                emit_co_pair(4)
                yield "co2"
                emit_co_pair(6)
                yield "co3"

            # interleaved emission: batch 1's input DMAs and head overlap
            # batch 0's compute; see docstring.
            gens = [emit_batch(bi) for bi in range(NB)]
            if NB == 2:
                b0, b1 = gens

                def step(g_):
                    return next(g_, None)

                step(b0)  # b0 dma_a (q + first passage half)
                emit_const_loads()
                step(b0)  # b0 dma_b
                step(b1)  # b1 dma_a (all inputs lead the SP queue)
                step(b1)  # b1 dma_b
                step(b0)  # b0 head
                step(b0)  # b0 g0s
                step(b1)  # b1 head (Pool/DVE work under b0's exps)
                step(b0)  # b0 g0a
                step(b0)  # b0 g1s
                step(b0)  # b0 hph (releases b0's mi bank)
                step(b1)  # b1 g0s
                step(b0)  # b0 aq
                step(b1)  # b1 g0a
                step(b0)  # b0 apco1
                step(b1)  # b1 g1s
                step(b0)  # b0 apco2
                step(b1)  # b1 hph
                step(b0)  # b0 co2
                step(b0)  # b0 co3
                step(b1)  # b1 aq
                step(b1)  # b1 apco1
                step(b1)  # b1 apco2
                step(b1)  # b1 co2
                step(b1)  # b1 co3
                for g_ in gens:
                    for _ in g_:
                        pass
            else:
                emit_const_loads()
                for g_ in gens:
                    for _ in g_:
                        pass

    return nc


_nc_cache = None


def kernel(passage, question, passage_mask, question_mask, W, b):
    global _nc_cache
    _install_bir_wait_split()
    if _nc_cache is None:
        _nc_cache = build_nc()
    nc = _nc_cache

    passage = np.ascontiguousarray(passage, dtype=np.float32)
    question = np.ascontiguousarray(question, dtype=np.float32)
    passage_mask = np.ascontiguousarray(passage_mask, dtype=np.int32)
    question_mask = np.ascontiguousarray(question_mask, dtype=np.int32)
    W = np.ascontiguousarray(W, dtype=np.float32)
    b = np.ascontiguousarray(b, dtype=np.float32)

    in_maps = []
    for c in range(N_CORES):
        s = slice(c * NB, (c + 1) * NB)
        in_maps.append(
            {
                "passage": passage[s],
                "question": question[s],
                "passage_mask": passage_mask[s],
                "question_mask": question_mask[s],
                "W": W,
                "b": b,
            }
        )
    res = run_bass_kernel_spmd(nc, in_maps, list(range(N_CORES)))
    p2q = np.concatenate(
        [np.asarray(r["p2q"], dtype=np.float32) for r in res.results], axis=0
    )
    coatt = np.concatenate(
        [np.asarray(r["coatt"], dtype=np.float32) for r in res.results], axis=0
    )
    return p2q, coatt
